# revision 1
# baseline (speedup 1.0000x reference)
"""Trainium2 Bass kernel for a 4-D stride-1 ConvTranspose
(B=2, C=32->32, S=16^4, K=3^4, output 18^4) -- fp8 DoubleRow edition.

Distribution: 8 cores = batch (2) x input-row chunks (p0 in 4 chunks of 4 rows).

Per core the tensor engine computes, in PSUM per (p0, q1) tile,
  z[(k0,o), p0, q1, q2, q3] = sum_{i,k3, valid k1, k2} w[i,o,k0,k1,k2,k3]
                              * x[i, p0, q1-k1, q2-k2, q3-k3]
with the contraction packed as 128 partitions x 2 DoubleRow pairs of
fp8e4m3 operands.  Precision: x and w are each split hi+lo (xa+xb, wa+wb
with the lo part the quantized residual of the hi fp8 cast); the matmuls
cover the hi*hi product plus correction terms per SCHEME:

  SCHEME 15: A=wa*xa, B=wa*xb, C=wb*xa fully (5 DR per k2-group) ~3e-3
  SCHEME 12: as 15 but C2 dropped for the k1=2 tap (4 DR)          ~1.0e-2
  SCHEME  9: A, B0, B1, C fully; B2 dropped (3 DR, 1 DR per tap)   ~1.5e-2

Partition layouts of the moving buffers (blocks of 32 channels):
  X1 = [xa(k3=0), xa(k3=1), xa(k3=2), xb(k3=0)]           (all schemes)
  X2 = [xa(k3=0), xa(k3=1), xb(k3=1), xb(k3=2)]           (schemes 15/12)
  X2 = [xa(k3=0), xa(k3=1), xa(k3=2), xb(k3=1)]           (scheme 9)
Coefficient halves per tap t=(k1,k2):
  H1(X1) = [wa0, wa1, wa2, wa0]                            (A + B0)
  H2(X2) = [wb0, wb1, wa1, wa2]   (15/12: C0, C1, B1, B2)
  H2(X2) = [wb0, wb1, wb2, wa1]   (9:     C0, C1, C2, B1)
  H3(X1) = [0,   0,   wb2, wb0]   (C2, D0)
  H4(X2) = [0,   0,   wb1, wb2]   (D1, D2; pads odd half counts)
One DoubleRow matmul carries two halves; the moving pair axis is either
the X1/X2 buffer axis (same p1 row) or two adjacent p1 rows within X1
(adjacent k1 taps), both plain rectangular AP slices.  k2 is handled by
writing each tap's 16-row window of the PSUM q2 axis; k0 stays packed in
the output partitions (96 = 3x32).

z is evacuated PSUM -> SBUF bf16 (DVE/ACT alternating) and DMA'd out in
3-q1 chunks; the host does the k0 -> q0 fold (which also realizes the
chunk-seam accumulation), rescales, and adds the bias.
"""

import os
import numpy as np
import ml_dtypes

B, CIN, COUT = 2, 32, 32
S, KT = 16, 3
Q = S + KT - 1            # 18
P0C = 4                   # input rows per core
NCORES = 8
FREE = Q * Q              # 324 (q2,q3)
ZROW = Q * FREE           # 5832 z elements per p0-row per partition
RB = S * Q                # 288: one p1 row = (p2, q3) flattened
XROW = S * RB             # 4608 elements per p0-row per partition per buffer
NBUF = 2

SCHEME = int(os.environ.get("KSCHEME", "9"))


def _order():
    """Stationary slot order; early-needed (low q1) slots first."""
    if SCHEME == 9:
        return (
            [("a", 0, k2) for k2 in range(3)]
            + [("a", 1, k2) for k2 in range(3)]
            + [("a", 2, k2) for k2 in range(3)]
        ), 6, 9
    order = (
        [("a", 0, k2) for k2 in range(3)]
        + [("l", 0, k2) for k2 in range(3)]
        + [("a", 1, k2) for k2 in range(3)]
        + [("b", (0, 1), k2) for k2 in range(3)]
        + [("a", 2, k2) for k2 in range(3)]
        + [("l", 2, k2) for k2 in range(3)]
        + [("b", (1, 2), k2) for k2 in range(3)]  # first needed at q1=16
    )
    return order, 6, 18


_ORDER, WF1, WF2 = _order()
IDX = {key: i for i, key in enumerate(_ORDER)}
NDR = len(_ORDER)

_CACHE = {}


def _tile_mms(q1):
    """DoubleRow matmul list for one (p0, q1) tile: (slot, kind, p1, k2)."""
    ks1 = [k1 for k1 in range(KT) if 0 <= q1 - k1 < S]
    mms = []
    for k2 in range(KT):
        for k1 in ks1:
            mms.append((IDX[("a", k1, k2)], "buf", q1 - k1, k2))
        if SCHEME == 9:
            continue
        if len(ks1) == 3:
            mms.append((IDX[("b", (0, 1), k2)], "p1", q1 - 1, k2))
            if SCHEME == 15:
                mms.append((IDX[("l", 2, k2)], "buf", q1 - 2, k2))
        elif len(ks1) == 2:
            ka, kb = ks1
            mms.append((IDX[("b", (ka, kb), k2)], "p1", q1 - kb, k2))
        else:
            k1 = ks1[0]
            mms.append((IDX[("l", k1, k2)], "buf", q1 - k1, k2))
    return mms


def _build_nc():
    import concourse.bass as bass
    import concourse.mybir as mybir
    from concourse.tile import TileContext

    f8 = mybir.dt.float8e4
    bf16 = mybir.dt.bfloat16
    f32 = mybir.dt.float32
    DRM = mybir.MatmulPerfMode.DoubleRow

    nc = bass.Bass()
    xs_d = nc.declare_dram_parameter("xs", [128, NBUF * P0C * XROW], f8, isOutput=False)
    wf_d = nc.declare_dram_parameter("wf", [128, NDR * 192], f8, isOutput=False)
    z_d = nc.declare_dram_parameter("z", [96, P0C * ZROW], bf16, isOutput=True)

    with TileContext(nc) as tc:
        with (
            tc.tile_pool(name="const", bufs=1) as cpool,
            tc.tile_pool(name="xsp", bufs=1) as xspool,
            tc.tile_pool(name="zcp", bufs=24) as zcpool,
            tc.tile_pool(name="zpsp", bufs=8, space="PSUM") as zps_pool,
        ):
            wf_sb = cpool.tile([128, NDR * 192], f8)
            xs_sb = xspool.tile([128, NBUF * P0C * XROW], f8)
            HCH = 4 * RB  # 4 p1 rows per chunk
            # Startup: the q1<2 wf slots and X2 chunk0 on sync/HWDGE while X1
            # chunk0 loads through the gpsimd queue in parallel; the rest of
            # wf follows, then the remaining chunks (X1 on gpsimd, X2 on
            # sync).
            nc.sync.dma_start(out=wf_sb[:, : WF1 * 192], in_=wf_d[:, : WF1 * 192])
            nc.gpsimd.dma_start(out=xs_sb[:, 0:HCH], in_=xs_d[:, 0:HCH])
            o2c0 = P0C * XROW
            nc.sync.dma_start(
                out=xs_sb[:, o2c0:o2c0 + HCH], in_=xs_d[:, o2c0:o2c0 + HCH]
            )
            nc.sync.dma_start(out=wf_sb[:, WF1 * 192:], in_=wf_d[:, WF1 * 192:])
            for p0 in range(P0C):
                for h in range(4):
                    if p0 == 0 and h == 0:
                        continue
                    o1 = p0 * XROW + h * HCH
                    nc.gpsimd.dma_start(
                        out=xs_sb[:, o1:o1 + HCH], in_=xs_d[:, o1:o1 + HCH]
                    )
                    o2 = P0C * XROW + o1
                    nc.sync.dma_start(
                        out=xs_sb[:, o2:o2 + HCH], in_=xs_d[:, o2:o2 + HCH]
                    )

            # [128, buf(2), p0(4), p1(16), (p2,q3)=288]
            xv = xs_sb.rearrange(
                "p (u r a bc) -> p u r a bc", u=NBUF, r=P0C, a=S, bc=RB
            )
            wv = wf_sb.rearrange("p (d two m) -> p d two m", d=NDR, two=2, m=96)

            for p0 in range(P0C):
                # For the last row, compute q1=15 LAST: the kernel's final
                # z DMA is then a single small tile whose SP/HWDGE queue
                # predecessors ([16,17]) were dispatched while it computed.
                order = (
                    list(range(Q)) if p0 < P0C - 1
                    else list(range(Q - 3)) + [Q - 2, Q - 1, Q - 3]
                )
                for q1 in order:
                    z_ps = zps_pool.tile([96, FREE], f32)
                    z_pv = z_ps.rearrange("p (a b) -> p a b", a=Q, b=Q)
                    mms = _tile_mms(q1)
                    n = len(mms)
                    for j, (slot, kind, p1, k2) in enumerate(mms):
                        if kind == "buf":
                            rhs = xv[:, :, p0, p1, :]
                        else:
                            rhs = xv[:, 0, p0, p1:p1 + 2, :]
                        nc.tensor.matmul(
                            z_pv[:, k2:k2 + S, :],
                            wv[:, slot],
                            rhs,
                            start=(j == 0),
                            stop=(j == n - 1),
                            perf_mode=DRM,
                        )
                    ch, within = divmod(q1, 3)
                    last_row = p0 == P0C - 1 and ch == Q // 3 - 1
                    first_in_chunk = within == 0 if not last_row else q1 == Q - 2
                    if first_in_chunk:
                        zc = zcpool.tile([96, 3 * FREE], bf16)
                    dst = zc[:, within * FREE:(within + 1) * FREE]
                    if last_row and q1 == Q - 3:
                        # final tile: split the copy so both engines halve it
                        h = FREE // 2
                        nc.vector.tensor_copy(out=dst[:, :h], in_=z_ps[:, :h])
                        nc.scalar.copy(dst[:, h:], z_ps[:, h:])
                    elif q1 % 2 == 1:
                        nc.vector.tensor_copy(out=dst, in_=z_ps[:, :])
                    else:
                        nc.scalar.copy(dst, z_ps[:, :])
                    off0 = (p0 * Q + ch * 3) * FREE
                    if last_row and q1 == Q - 1:
                        # [16,17] flushed while q1=15 still computes
                        nc.sync.dma_start(
                            out=z_d[:, off0 + FREE:off0 + 3 * FREE],
                            in_=zc[:, FREE:3 * FREE],
                        )
                    elif last_row and q1 == Q - 3:
                        nc.sync.dma_start(out=z_d[:, off0:off0 + FREE], in_=dst)
                    elif within == 2:
                        nc.sync.dma_start(
                            out=z_d[:, off0:off0 + 3 * FREE], in_=zc[:, :]
                        )

    _split_drain_waits(nc)
    return nc


def _split_drain_waits(nc, max_waits=1):
    """walrus CoreV3 codegen rejects instructions carrying multiple sem waits
    ("Too many sync wait commands"); hoist extras onto preceding
    single-wait NoOp instructions on the same engine."""
    import concourse.mybir as mybir

    for f in nc.m.functions:
        for b in f.blocks:
            out = []
            changed = False
            for inst in b.instructions:
                si = inst.sync_info
                if si is not None and len(si.on_wait) > max_waits:
                    waits = list(si.on_wait)
                    for k, w in enumerate(waits[:-max_waits]):
                        nd = mybir.InstNoOp(
                            name=f"{inst.name}-wsplit{k}", ins=[], outs=[]
                        )
                        nd.engine = inst.engine
                        nd.sync_info = mybir.SyncInfo(on_wait=[w], on_update=[])
                        nc.register_instruction(nd, overwrite=True)
                        out.append(nd)
                    inst.sync_info = mybir.SyncInfo(
                        on_wait=waits[-max_waits:], on_update=list(si.on_update)
                    )
                    changed = True
                out.append(inst)
            if changed:
                b.instructions = out


def _prep_host(x, weight):
    """Host-side fp8 split + layouts. Returns (xs_cores, wf, scale)."""
    f8 = ml_dtypes.float8_e4m3fn
    f32 = np.float32

    sx = f32(1.0 / max(x.std(), 1e-30))
    sw = f32(1.0 / max(weight.std(), 1e-30))
    xn = (x * sx).astype(f32)
    wn = (weight * sw).astype(f32)

    xa = xn.astype(f8)
    xb = (xn - xa.astype(f32)).astype(f8)

    # shifted copies: sh[k3, i, p0, p1, p2, q3] = t[i, p0, p1, p2, q3-k3]
    def shift(t):  # [B, 32, 16,16,16,16] -> [B, 3, 32, 16,16,16,18]
        out = np.zeros((B, KT, CIN, S, S, S, Q), dtype=f8)
        for k3 in range(KT):
            out[:, k3, :, :, :, :, k3:k3 + S] = t
        return out

    sa = shift(xa)
    sb = shift(xb)
    x1 = np.concatenate([sa[:, 0], sa[:, 1], sa[:, 2], sb[:, 0]], axis=1)
    if SCHEME == 9:
        x2 = np.concatenate([sa[:, 0], sa[:, 1], sa[:, 2], sb[:, 1]], axis=1)
    else:
        x2 = np.concatenate([sa[:, 0], sa[:, 1], sb[:, 1], sb[:, 2]], axis=1)
    xs_full = np.stack([x1, x2], axis=2)  # [B, 128, buf, p0(16), p1, p2, q3]

    xs_cores = []
    for core in range(NCORES):
        n, c = divmod(core, P0C)
        blk = xs_full[n, :, :, P0C * c:P0C * (c + 1)]
        xs_cores.append(np.ascontiguousarray(blk).reshape(128, NBUF * P0C * XROW))

    # weight blocks wt[(k3,i), (k0,o), k1, k2]
    wt = np.ascontiguousarray(
        wn.transpose(5, 0, 2, 1, 3, 4)  # k3, i, k0, o, k1, k2
    ).reshape(96, 96, KT, KT).astype(f32)
    wa = wt.astype(f8)
    wb = (wt - wa.astype(f32)).astype(f8)

    def blks(t, k1, k2):
        m = t[:, :, k1, k2]
        return m[0:32], m[32:64], m[64:96]

    wf = np.zeros((128, NDR, 2, 96), dtype=f8)
    for key, d in IDX.items():
        kind = key[0]
        if kind == "a":
            _, k1, k2 = key
            a0, a1, a2 = blks(wa, k1, k2)
            b0, b1, b2 = blks(wb, k1, k2)
            wf[:, d, 0] = np.concatenate([a0, a1, a2, a0])        # H1 on X1
            if SCHEME == 9:
                wf[:, d, 1] = np.concatenate([b0, b1, b2, a1])    # H2' on X2
            else:
                wf[:, d, 1] = np.concatenate([b0, b1, a1, a2])    # H2 on X2
        elif kind == "l":
            _, k1, k2 = key
            a0, a1, a2 = blks(wa, k1, k2)
            b0, b1, b2 = blks(wb, k1, k2)
            zeros = np.zeros_like(b0)
            wf[:, d, 0] = np.concatenate([zeros, zeros, b2, b0])  # H3 on X1
            wf[:, d, 1] = np.concatenate([zeros, zeros, b1, b2])  # H4 on X2
        else:
            _, (ka, kb), k2 = key
            za = np.zeros((32, 96), dtype=f8)
            for j, k1 in enumerate((kb, ka)):  # pair j=0 is the smaller p1
                b0, b1, b2 = blks(wb, k1, k2)
                wf[:, d, j] = np.concatenate([za, za, b2, b0])    # H3
    wf = np.ascontiguousarray(wf).reshape(128, NDR * 192)

    return xs_cores, wf, f32(1.0) / (sx * sw)


def _make_in_maps(np_inputs):
    xs_cores, wf, _ = _prep_host(
        np.asarray(np_inputs["x"], np.float32),
        np.asarray(np_inputs["weight"], np.float32),
    )
    return [{"xs": xs_cores[core], "wf": wf} for core in range(NCORES)]


def kernel(x, weight, bias):
    from concourse.bass_utils import run_bass_kernel_spmd

    x = np.asarray(x, np.float32)
    weight = np.asarray(weight, np.float32)
    bias = np.asarray(bias, np.float32)

    if "nc" not in _CACHE:
        _CACHE["nc"] = _build_nc()
    nc = _CACHE["nc"]

    xs_cores, wf, scale = _prep_host(x, weight)
    in_maps = [{"xs": xs_cores[core], "wf": wf} for core in range(NCORES)]
    res = run_bass_kernel_spmd(nc, in_maps, list(range(NCORES)))

    y = np.zeros((B, COUT, Q, Q, Q, Q), np.float32)
    for core in range(NCORES):
        n, c = divmod(core, P0C)
        zc = res.results[core]["z"].astype(np.float32).reshape(
            KT, COUT, P0C, Q, Q, Q
        )
        for k0 in range(KT):
            y[n, :, P0C * c + k0:P0C * c + k0 + P0C] += zc[k0]
    y *= scale
    y += bias.reshape(1, -1, 1, 1, 1, 1)
    return y

